# revision 12
# baseline (speedup 1.0000x reference)
"""Trainium2 Bass kernel for the batched differentiable EKF.

B=8192 rows x T=2048 sequential EKF steps (2-state KF, scalar obs).
Output [B, T, 2] f32.

Design (v5):
- Data parallel: 1024 rows/core over 8 cores; rows -> 8 groups x 128
  partitions.
- Time parallel per core: T split into C=39 chunks of L=52 steps with a
  W=20-step warmup from a cold init (x=[z,dz], P=I). Chunk 0's warmup is
  the true filter start, so its warmup outputs are kept. Warmup
  truncation error ~3.5e-3 rel vs the 2e-2 gate (numpy prototype).
- fp16 everywhere: DVE tensor_tensor runs in 2x mode for 2-byte packed
  dtypes (0.52 ns/elem), and fp16's 10 mantissa bits keep the noise
  floor at ~1.7e-3 rel (bf16 was 1.4e-2 - too hot).
- Host pre-gathers inputs into the exact SBUF slab layout
  [slab][part][step][lane] (lane = group*C + chunk) so every DMA is a
  fully contiguous 128-descriptor transfer; host scatters outputs back.
- The whole Riccati recurrence (t1..p11') stays DVE-local so the
  step-to-step dependency never crosses engines; Pool gets only
  chain-terminal ops (pq, pp11, x0', k1y, x1'); ACT computes sq01
  (Square) plus the bulk input derivation, nibbled one range per step
  so it never head-blocks sq01 on the in-order ACT queue.
- x-part lags the P-part by XDELAY steps so two independent dependency
  chains keep the engines fed.
"""

import numpy as np

import concourse.bass as bass
import concourse.bacc as bacc
import concourse.mybir as mybir
import concourse.tile as tile
from concourse.dve_ops import RECIP_APPROX_FAST_CONSTS, RECIPROCAL_APPROX_FAST

F16 = mybir.dt.float16
F32 = mybir.dt.float32
ALU = mybir.AluOpType
ACT = mybir.ActivationFunctionType
PART = 128

# geometry
B, T = 8192, 2048
NCORES = 8
B_LOC = B // NCORES          # 1024
G = B_LOC // PART            # 8
W, L, C = 20, 52, 39         # warmup, chunk len, chunks; C*L + W == T
GC = G * C                   # 312 lanes per partition
STEPS = W + L                # 72
NS = 12                      # steps per slab
NSLAB = STEPS // NS          # 6
XDELAY = 4

assert C * L + W == T and NSLAB * NS == STEPS


def build_core_kernel():
    NSGC = NS * GC
    nc = bacc.Bacc("TRN2", target_bir_lowering=False, debug=False)
    z_h = nc.dram_tensor("z", [NSLAB, PART, NSGC], F16, kind="ExternalInput")
    h_h = nc.dram_tensor("h", [NSLAB, PART, NSGC], F16, kind="ExternalInput")
    v_h = nc.dram_tensor("v", [NSLAB, PART, NSGC], F16, kind="ExternalInput")
    out_h = nc.dram_tensor(
        "out", [NSLAB, PART, NS * 2 * GC], F16, kind="ExternalOutput"
    )

    rc = RECIP_APPROX_FAST_CONSTS

    def dram_ap(handle, si, width):
        return bass.AP(
            tensor=handle, offset=si * PART * width, ap=[[width, PART], [1, width]]
        )

    with tile.TileContext(nc) as tc:
        with (
            tc.tile_pool(name="io", bufs=2) as iop,
            tc.tile_pool(name="ost", bufs=3) as ostp,
            tc.tile_pool(name="st", bufs=4) as stp,
            tc.tile_pool(name="uk", bufs=XDELAY + 3) as ukp,
            tc.tile_pool(name="ini", bufs=1) as inip,
        ):
            V = nc.vector
            GP = nc.gpsimd
            SC = nc.scalar

            p00_i = inip.tile([PART, GC], F16, tag="p00i")
            p01_i = inip.tile([PART, GC], F16, tag="p01i")
            p11_i = inip.tile([PART, GC], F16, tag="p11i")
            x0_i = inip.tile([PART, GC], F16, tag="x0i")
            x1_i = inip.tile([PART, GC], F16, tag="x1i")
            bias_m5 = inip.tile([PART, 1], F32, tag="bm5")
            bias_m1 = inip.tile([PART, 1], F32, tag="bm1")
            GP.memset(bias_m5[:], -5.0)
            GP.memset(bias_m1[:], -1.0)
            GP.memset(p00_i[:], 1.0)
            GP.memset(p01_i[:], 0.0)
            GP.memset(p11_i[:], 1.0)

            slab_ctx = {}
            pprev = {}
            xprev = {}
            kctx = {}

            def load_slab_dma(si):
                z_sl = iop.tile([PART, NSGC], F16, tag="z")
                h_sl = iop.tile([PART, NSGC], F16, tag="h")
                v_sl = iop.tile([PART, NSGC], F16, tag="v")
                a_sl = iop.tile([PART, NSGC], F16, tag="a")
                s_sl = iop.tile([PART, NSGC], F16, tag="scl")
                q_sl = iop.tile([PART, NSGC], F16, tag="qq")
                o_sl = ostp.tile([PART, NS * 2 * GC], F16, tag="o")
                nc.sync.dma_start(z_sl[:], dram_ap(z_h, si, NSGC))
                nc.sync.dma_start(h_sl[:], dram_ap(h_h, si, NSGC))
                nc.sync.dma_start(v_sl[:], dram_ap(v_h, si, NSGC))
                slab_ctx[si] = dict(
                    h_sl=h_sl, v_sl=v_sl, a_sl=a_sl, s_sl=s_sl, q_sl=q_sl,
                    zv=z_sl[:].rearrange("p (s gc) -> p s gc", s=NS),
                    av=a_sl[:].rearrange("p (s gc) -> p s gc", s=NS),
                    sv=s_sl[:].rearrange("p (s gc) -> p s gc", s=NS),
                    qv=q_sl[:].rearrange("p (s gc) -> p s gc", s=NS),
                    o_sl=o_sl,
                    ov=o_sl[:].rearrange(
                        "p (s two gc) -> p s two gc", s=NS, two=2
                    ),
                )
                return slab_ctx[si]

            def emit_bulk(si, rng):
                """ACT derivation for one range of slab si:
                a = 0.5 + 0.5*sigmoid(10h-5); t = relu(100v-1);
                qq = 0.1t + 0.1; scl = t + 1."""
                sl = slab_ctx[si]
                cs = rng
                h_sl, v_sl = sl["h_sl"], sl["v_sl"]
                a_sl, s_sl, q_sl = sl["a_sl"], sl["s_sl"], sl["q_sl"]
                SC.activation(
                    a_sl[:][:, cs], h_sl[:][:, cs], ACT.Sigmoid,
                    bias=bias_m5[:], scale=10.0,
                )
                SC.activation(
                    a_sl[:][:, cs], a_sl[:][:, cs], ACT.Copy,
                    bias=0.5, scale=0.5,
                )
                SC.activation(
                    s_sl[:][:, cs], v_sl[:][:, cs], ACT.Relu,
                    bias=bias_m1[:], scale=100.0,
                )
                SC.activation(
                    q_sl[:][:, cs], s_sl[:][:, cs], ACT.Copy,
                    bias=0.1, scale=0.1,
                )
                SC.activation(
                    s_sl[:][:, cs], s_sl[:][:, cs], ACT.Copy,
                    bias=1.0, scale=1.0,
                )

            def emit_p_head(gs):
                si, s = divmod(gs, NS)
                sl = slab_ctx[si]
                A = sl["av"][:, s]
                QQ = sl["qv"][:, s]
                p00p, p01p, p11p = pprev["p00"], pprev["p01"], pprev["p11"]

                pq = stp.tile([PART, GC], F16, tag="pq")
                pp11 = stp.tile([PART, GC], F16, tag="pp11")
                GP.tensor_tensor(out=pq[:], in0=QQ, in1=p00p, op=ALU.add)
                GP.tensor_tensor(out=pp11[:], in0=QQ, in1=p11p, op=ALU.add)

                t1 = stp.tile([PART, GC], F16, tag="t1")
                pp01 = stp.tile([PART, GC], F16, tag="pp01")
                g2 = stp.tile([PART, GC], F16, tag="g2")
                m = stp.tile([PART, GC], F16, tag="m")
                sq01 = stp.tile([PART, GC], F16, tag="sq01")
                V.tensor_tensor(out=t1[:], in0=A, in1=p11p, op=ALU.mult)
                V.tensor_tensor(out=pp01[:], in0=p01p, in1=t1[:], op=ALU.add)
                SC.activation(sq01[:], pp01[:], ACT.Square)
                V.tensor_tensor(out=g2[:], in0=pp01[:], in1=p01p, op=ALU.add)
                V.tensor_tensor(out=m[:], in0=A, in1=g2[:], op=ALU.mult)
                return dict(sl=sl, s=s, pq=pq, pp11=pp11, pp01=pp01,
                            m=m, sq01=sq01)

            def emit_p_tail(gs, h):
                sl, s = h["sl"], h["s"]
                SCL = sl["sv"][:, s]
                pq, pp11, pp01, m, sq01 = (
                    h["pq"], h["pp11"], h["pp01"], h["m"], h["sq01"]
                )
                pp00 = stp.tile([PART, GC], F16, tag="pp00")
                S = stp.tile([PART, GC], F16, tag="S")
                r = stp.tile([PART, GC], F16, tag="r")
                u = ukp.tile([PART, GC], F16, tag="u")
                k1 = ukp.tile([PART, GC], F16, tag="k1")
                p00n = stp.tile([PART, GC], F16, tag="p00")
                p01n = stp.tile([PART, GC], F16, tag="p01")
                t3 = stp.tile([PART, GC], F16, tag="t3")
                p11n = stp.tile([PART, GC], F16, tag="p11")
                V.tensor_tensor(out=pp00[:], in0=pq[:], in1=m[:], op=ALU.add)
                V.tensor_tensor(out=S[:], in0=pp00[:], in1=SCL, op=ALU.add)
                V._custom_dve(
                    RECIPROCAL_APPROX_FAST, out=r[:], in0=S[:],
                    s0=rc["s0"], s1=rc["s1"], imm2=rc["imm2"],
                )
                V.tensor_tensor(out=u[:], in0=SCL, in1=r[:], op=ALU.mult)
                V.tensor_tensor(out=k1[:], in0=r[:], in1=pp01[:], op=ALU.mult)
                V.tensor_tensor(out=p00n[:], in0=u[:], in1=pp00[:], op=ALU.mult)
                V.tensor_tensor(out=p01n[:], in0=SCL, in1=k1[:], op=ALU.mult)
                V.tensor_tensor(out=t3[:], in0=sq01[:], in1=r[:], op=ALU.mult)
                V.tensor_tensor(
                    out=p11n[:], in0=pp11[:], in1=t3[:], op=ALU.subtract
                )
                pprev.update(p00=p00n[:], p01=p01n[:], p11=p11n[:])
                kctx[gs] = (u, k1)

            def emit_x(gs):
                si, s = divmod(gs, NS)
                sl = slab_ctx[si]
                Z = sl["zv"][:, s]
                A = sl["av"][:, s]
                ov = sl["ov"]
                u, k1 = kctx.pop(gs)
                x0p, x1p = xprev["x0"], xprev["x1"]

                t4 = stp.tile([PART, GC], F16, tag="t4")
                xp = stp.tile([PART, GC], F16, tag="xp")
                y = stp.tile([PART, GC], F16, tag="y")
                uy = stp.tile([PART, GC], F16, tag="uy")
                k1y = stp.tile([PART, GC], F16, tag="k1y")
                V.tensor_tensor(out=t4[:], in0=A, in1=x1p, op=ALU.mult)
                V.tensor_tensor(out=xp[:], in0=x0p, in1=t4[:], op=ALU.add)
                V.tensor_tensor(out=y[:], in0=Z, in1=xp[:], op=ALU.subtract)
                V.tensor_tensor(out=uy[:], in0=u[:], in1=y[:], op=ALU.mult)
                GP.tensor_tensor(
                    out=ov[:, s, 0], in0=Z, in1=uy[:], op=ALU.subtract
                )
                GP.tensor_tensor(out=k1y[:], in0=k1[:], in1=y[:], op=ALU.mult)
                GP.tensor_tensor(
                    out=ov[:, s, 1], in0=x1p, in1=k1y[:], op=ALU.add
                )
                xprev.update(x0=ov[:, s, 0], x1=ov[:, s, 1])

                if s == NS - 1:
                    nc.sync.dma_start(
                        dram_ap(out_h, si, NS * 2 * GC), sl["o_sl"][:]
                    )

            nkr = NSGC // NS  # bulk range size: one step's worth
            for gs in range(STEPS + XDELAY):
                if gs < STEPS:
                    si, s = divmod(gs, NS)
                    if gs == 0:
                        ctx = load_slab_dma(0)
                        emit_bulk(0, slice(0, 2 * nkr))
                        zv = ctx["zv"]
                        V.tensor_copy(x0_i[:], zv[:, 0])
                        V.tensor_tensor(
                            out=x1_i[:], in0=zv[:, 1], in1=zv[:, 0],
                            op=ALU.subtract,
                        )
                        pprev.update(
                            p00=p00_i[:], p01=p01_i[:], p11=p11_i[:]
                        )
                        xprev.update(x0=x0_i[:], x1=x1_i[:])
                    if s == 4 and si + 1 < NSLAB:
                        load_slab_dma(si + 1)
                    h = emit_p_head(gs)
                    # nibble the next slab's (or slab 0's tail) derivation
                    if si == 0 and gs < NS - 2 and gs >= 0:
                        emit_bulk(0, slice((gs + 2) * nkr, (gs + 3) * nkr))
                    if si + 1 < NSLAB and s >= 4:
                        r0 = (s - 4) * NS // (NS - 4)
                        r1 = (s - 3) * NS // (NS - 4)
                        if r1 > r0:
                            emit_bulk(
                                si + 1, slice(r0 * nkr, r1 * nkr)
                            )
                    if gs - XDELAY >= 0:
                        emit_x(gs - XDELAY)
                    emit_p_tail(gs, h)
                else:
                    emit_x(gs - XDELAY)
    nc.compile()
    return nc


_nc_cache = {}


def _get_nc():
    if "nc" not in _nc_cache:
        _nc_cache["nc"] = build_core_kernel()
    return _nc_cache["nc"]


# host-side gather/scatter index: col of (step gs, chunk c) = c*L + gs
_COLS = (np.arange(C)[None, :] * L + np.arange(STEPS)[:, None])  # [STEPS, C]


def _stage_input(arr):
    """[B_LOC, T] f32 -> [NSLAB, PART, NS*GC] f16 in slab layout."""
    xf = arr.astype(np.float16)
    g3 = xf.reshape(G, PART, T)[:, :, _COLS]          # [G, P, STEPS, C]
    g4 = np.transpose(g3, (2, 1, 0, 3))                # [STEPS, P, G, C]
    st = g4.reshape(NSLAB, NS, PART, GC)
    st = np.transpose(st, (0, 2, 1, 3)).reshape(NSLAB, PART, NS * GC)
    return np.ascontiguousarray(st)


def _unstage_output(dev_out):
    """[NSLAB, PART, NS*2*GC] f16 -> [B_LOC, T, 2] f32."""
    o = np.asarray(dev_out).reshape(NSLAB, PART, NS, 2, G, C)
    o = np.transpose(o, (4, 1, 0, 2, 5, 3))            # [G, P, NSLAB, NS, C, 2]
    o = o.reshape(B_LOC, STEPS, C, 2).astype(np.float32)
    res = np.empty((B_LOC, T, 2), np.float32)
    res[:, _COLS[:, 0], :] = o[:, :, 0, :]
    res[:, _COLS[W:, 1:], :] = o[:, W:, 1:, :]
    return res


def kernel(price: np.ndarray, hurst: np.ndarray, vol_sigma: np.ndarray) -> np.ndarray:
    from concourse import bass_utils

    price = np.ascontiguousarray(price, dtype=np.float32)
    hurst = np.ascontiguousarray(hurst, dtype=np.float32)
    vol_sigma = np.ascontiguousarray(vol_sigma, dtype=np.float32)
    nc = _get_nc()
    in_maps = []
    for k in range(NCORES):
        sl = slice(k * B_LOC, (k + 1) * B_LOC)
        in_maps.append(
            {
                "z": _stage_input(price[sl]),
                "h": _stage_input(hurst[sl]),
                "v": _stage_input(vol_sigma[sl]),
            }
        )
    res = bass_utils.run_bass_kernel_spmd(
        nc, in_maps, core_ids=list(range(NCORES))
    )
    return np.concatenate(
        [_unstage_output(r["out"]) for r in res.results], axis=0
    )
